# revision 5
# baseline (speedup 1.0000x reference)
import os
import sys
import time
import numpy as np

# nn_CRF loss on 8 NeuronCores: emissions [L,B,T], tags/qmask/mask [L,B],
# transitions T/TxT.  Returns scalar f32 sum_b (gold-path score - logZ).
#
# Strategy (data-parallel over B):
#  - numerator (gold path score): cheap gathers, computed on host in numpy.
#  - denominator logZ: forward algorithm as a *linear-domain* scaled scan on
#    device.  alpha_{l+1} = (alpha_l @ exp(selfT)) * exp(em_{l+1}), with a
#    per-partition max-rescale every R steps; the max is stored to a column
#    of an SBUF tile and logged/accumulated on host afterwards.
#  - exp(em) is precomputed on host; each of the 8 cores holds its whole
#    [P, L*G*T] slice in SBUF as 32 distinct chunk tiles (no ring reuse,
#    so chunk DMAs carry no sync waits).
#  - each of the 8 cores handles B/8 = 256 sequences as [128 partitions, 2
#    groups]; host folds start_transitions into em[0], applies
#    end_transitions at the end, and sums.

L, B, T = 2048, 2048, 7
NCORES = 8
P = 128                       # partitions
G = (B // NCORES) // P        # 2 batch groups per core
CH = 64                       # scan steps per SBUF chunk tile
NCH = L // CH                 # 32 chunks
R = 8                         # renormalize every R steps
NREN = L // R - 1             # 255 renorms (none after the last step)
GT = G * T                    # 14
GTT = G * T * T               # 98

LAST_EXEC_NS = None


def _build_bass():
    import concourse.mybir as mybir
    from concourse.bacc import Bacc
    from concourse.tile import TileContext

    f32 = mybir.dt.float32
    Alu = mybir.AluOpType
    X = mybir.AxisListType.X

    # Bacc (not bass.Bass): its compile() runs generate_event_semaphores,
    # which splits multi-wait sync_info to satisfy the TRN2 1-wait-per-
    # instruction constraint — walrus rejects the module otherwise.
    nc = Bacc()
    em = nc.declare_dram_parameter("em", [NCH, P, CH * GT], f32, isOutput=False)
    e128 = nc.declare_dram_parameter("e128", [P, T * T], f32, isOutput=False)
    alpha_out = nc.declare_dram_parameter("alpha_out", [P, GT], f32, isOutput=True)
    smax_out = nc.declare_dram_parameter("smax_out", [P, NREN], f32, isOutput=True)

    with TileContext(nc) as tc:
        with (
            tc.tile_pool(name="const", bufs=1) as cpool,
            tc.tile_pool(name="emp", bufs=1) as empool,
            tc.tile_pool(name="work", bufs=2) as wpool,
        ):
            E = cpool.tile([P, T * T], f32, tag="E")
            nc.sync.dma_start(out=E[:, :], in_=e128[:, :])
            alpha = cpool.tile([P, GT], f32, tag="alpha")
            smx = cpool.tile([P, NREN], f32, tag="smx")

            # all 32 chunks live in SBUF simultaneously (114 KiB/partition):
            # distinct tiles, so no WAR hazards on the loads.
            emt = []
            for c in range(NCH):
                t_ = empool.tile([P, CH * GT], f32, tag=f"em{c}")
                nc.sync.dma_start(out=t_[:, :], in_=em[c, :, :])
                emt.append(t_)

            # 3-D APs only (walrus verifier limit for tensor_tensor): per-g views
            e_b = E[:, :].rearrange("p (j i) -> p j i", j=T)

            rt = None
            ren = 0
            for c in range(NCH):
                expem = emt[c]
                s0 = 0
                if c == 0:
                    nc.vector.tensor_copy(out=alpha[:, :], in_=expem[:, 0:GT])
                    s0 = 1
                for s in range(s0, CH):
                    step = c * CH + s
                    qf = wpool.tile([P, GTT], f32, tag="qf")
                    q = wpool.tile([P, GT], f32, tag="q")
                    for g in range(G):
                        a_b = alpha[:, g * T:(g + 1) * T].rearrange(
                            "p t -> p () t"
                        ).broadcast_to([P, T, T])
                        qf3 = qf[:, g * T * T:(g + 1) * T * T].rearrange(
                            "p (j i) -> p j i", j=T
                        )
                        if step % R == 0 and step > 0:
                            # fold in the pending 1/max rescale from the renorm
                            nc.vector.scalar_tensor_tensor(
                                out=qf3, in0=a_b, scalar=rt[:, 0:1], in1=e_b,
                                op0=Alu.mult, op1=Alu.mult,
                            )
                        else:
                            nc.vector.tensor_tensor(
                                out=qf3, in0=a_b, in1=e_b, op=Alu.mult
                            )
                    nc.vector.tensor_reduce(
                        out=q[:, :],
                        in_=qf[:, :].rearrange("p (gj i) -> p gj i", i=T),
                        axis=X, op=Alu.add,
                    )
                    nc.vector.tensor_tensor(
                        out=alpha[:, :], in0=q[:, :],
                        in1=expem[:, s * GT:(s + 1) * GT], op=Alu.mult,
                    )
                    if (step + 1) % R == 0 and step != L - 1:
                        rt = wpool.tile([P, 1], f32, tag="rt")
                        nc.vector.reduce_max(
                            out=smx[:, ren:ren + 1], in_=alpha[:, :], axis=X
                        )
                        nc.vector.reciprocal(out=rt[:, :], in_=smx[:, ren:ren + 1])
                        ren += 1
            assert ren == NREN
            nc.sync.dma_start(out=alpha_out[:, :], in_=alpha[:, :])
            nc.sync.dma_start(out=smax_out[:, :], in_=smx[:, :])
    nc.compile()
    return nc


def _device_logZ(emissions, start_transitions, end_transitions, self_transitions):
    """Returns logZ summed over all B, computed on 8 NeuronCores."""
    global LAST_EXEC_NS
    sys.path.insert(0, "/opt/trn_rl_repo")
    from concourse.bass_utils import run_bass_kernel_spmd

    t0 = time.time()
    # host relayout: em_dev[core, c, p, s*G*T + g*T + t] = em[c*CH+s, core*256+g*128+p, t]
    eml = np.asarray(emissions, dtype=np.float32).reshape(NCH, CH, NCORES, G, P, T)
    em_dev = np.ascontiguousarray(eml.transpose(2, 0, 4, 1, 3, 5))
    em_dev[:, 0, :, 0, :, :] += np.asarray(start_transitions, np.float32)[None, None, None, :]
    np.exp(em_dev, out=em_dev)
    em_dev = em_dev.reshape(NCORES, NCH, P, CH * GT)
    E_np = np.exp(np.asarray(self_transitions, np.float64)).astype(np.float32).T  # [j,i]
    E128 = np.broadcast_to(E_np.reshape(1, T * T), (P, T * T)).copy()
    t1 = time.time()

    nc = _build_bass()
    t2 = time.time()

    in_maps = [{"em": em_dev[i], "e128": E128} for i in range(NCORES)]
    # NTFF profiling under axon needs antenv.axon_hooks; only attempt a
    # traced run when it is importable (a failed traced run would force a
    # second full compile+execute).
    want_trace = False
    try:
        import antenv.axon_hooks  # noqa: F401
        want_trace = True
    except Exception:
        pass
    res = None
    if want_trace:
        try:
            res = run_bass_kernel_spmd(nc, in_maps, list(range(NCORES)),
                                       trace=True)
            LAST_EXEC_NS = res.exec_time_ns
        except Exception as e:
            print(f"[kernel] traced run failed ({e!r}); retrying without "
                  f"trace", file=sys.stderr)
            res = None
    if res is None:
        res = run_bass_kernel_spmd(nc, in_maps, list(range(NCORES)))
        LAST_EXEC_NS = None
    t3 = time.time()
    print(f"[kernel] relayout {t1-t0:.2f}s build {t2-t1:.2f}s run {t3-t2:.2f}s "
          f"exec_ns={LAST_EXEC_NS}", file=sys.stderr)

    logZ = 0.0
    eend = np.exp(np.asarray(end_transitions, np.float64))  # [t]
    for i in range(NCORES):
        af = res.results[i]["alpha_out"].astype(np.float64).reshape(P, G, T)
        sm = res.results[i]["smax_out"].astype(np.float64)  # [P, NREN]
        off = np.sum(np.log(sm), axis=1)                    # [P]
        logZ += np.sum(np.log(af @ eend) + off[:, None])
    return logZ


def _host_score(em, tags, qmask, mask_i, st, et, selfT, otherT):
    """Gold path score, summed over B (numpy, f32 gathers / f64 sums)."""
    contagion = qmask[1:] != qmask[:-1]
    em_tag = np.take_along_axis(em, tags[:, :, None], axis=2)[:, :, 0]   # [L,B] f32
    if contagion.any():
        trans_tag = np.where(contagion,
                             otherT[tags[:-1], tags[1:]],
                             selfT[tags[:-1], tags[1:]])
    else:
        trans_tag = selfT[tags[:-1], tags[1:]]
    score = np.sum(st[tags[0]], dtype=np.float64)
    score += np.sum(em_tag[0], dtype=np.float64)
    if np.all(mask_i[1:] != 0):
        score += np.sum(trans_tag, dtype=np.float64)
        score += np.sum(em_tag[1:], dtype=np.float64)
        score += np.sum(et[tags[-1]], dtype=np.float64)
    else:
        maskf = mask_i[1:].astype(np.float64)
        score += np.sum((trans_tag + em_tag[1:]) * maskf)
        seq_ends = mask_i.sum(axis=0) - 1
        score += np.sum(et[tags[seq_ends, np.arange(em.shape[1])]], dtype=np.float64)
    return float(score)


def _host_logZ_simple(em, st, et, selfT):
    """Vectorized scaled linear-domain scan (simple case: full mask, one
    speaker). f64, renorm every 16 steps."""
    Efwd = np.exp(np.asarray(selfT, np.float64))            # [i,j]
    e = np.exp(np.asarray(em, np.float64))                  # [L,B,T]
    alpha = np.exp(np.asarray(st, np.float64))[None, :] * e[0]
    off = np.zeros(alpha.shape[0])
    for l in range(1, em.shape[0]):
        alpha = (alpha @ Efwd) * e[l]
        if l % 16 == 0:
            m = alpha.max(axis=1)
            alpha /= m[:, None]
            off += np.log(m)
    fin = alpha * np.exp(np.asarray(et, np.float64))[None, :]
    return float(np.sum(np.log(fin.sum(axis=1)) + off))


def _host_logZ_general(em, qmask, mask_i, st, et, selfT, otherT):
    em = np.asarray(em, np.float64)
    contagion = qmask[1:] != qmask[:-1]
    any_cont = contagion.any(axis=1)
    all_mask = (mask_i != 0).all(axis=1)
    alpha = st[None, :] + em[0]
    for l in range(1, em.shape[0]):
        if any_cont[l - 1]:
            trans = np.where(contagion[l - 1][:, None, None], otherT[None], selfT[None])
            x = alpha[:, :, None] + trans
        else:
            x = alpha[:, :, None] + selfT[None]
        m = x.max(axis=1)
        new = np.log(np.exp(x - m[:, None, :]).sum(axis=1)) + m + em[l]
        if all_mask[l]:
            alpha = new
        else:
            alpha = np.where(mask_i[l][:, None] > 0, new, alpha)
    fin = alpha + et[None, :]
    mm = fin.max(axis=1)
    return float(np.sum(np.log(np.exp(fin - mm[:, None]).sum(axis=1)) + mm))


def kernel(emissions, tags, qmask, mask, start_transitions, end_transitions,
           self_transitions, other_transitions):
    emissions = np.asarray(emissions, dtype=np.float32)
    tags = np.asarray(tags)
    qmask = np.asarray(qmask)
    mask_i = np.asarray(mask)
    st = np.asarray(start_transitions, np.float64)
    et = np.asarray(end_transitions, np.float64)
    selfT = np.asarray(self_transitions, np.float64)
    otherT = np.asarray(other_transitions, np.float64)

    score = _host_score(emissions, tags, qmask, mask_i, st, et, selfT, otherT)

    simple = (not np.any(qmask[1:] != qmask[:-1])) and np.all(mask_i != 0)
    logZ = None
    if simple and emissions.shape == (L, B, T):
        try:
            logZ = _device_logZ(emissions, start_transitions, end_transitions,
                                self_transitions)
        except Exception as e:
            print(f"[kernel] device path failed ({e!r}); numpy fallback",
                  file=sys.stderr)
            logZ = None
    if logZ is None:
        if simple:
            logZ = _host_logZ_simple(emissions, st, et, selfT)
        else:
            logZ = _host_logZ_general(emissions, qmask, mask_i, st, et,
                                      selfT, otherT)

    return np.array(score - logZ, dtype=np.float32)


# revision 6
# speedup vs baseline: 9.8359x; 9.8359x over previous
import os
import sys
import time
import numpy as np

# nn_CRF loss on 8 NeuronCores: emissions [L,B,T], tags/qmask/mask [L,B],
# transitions T/TxT.  Returns scalar f32 sum_b (gold-path score - logZ).
#
# Strategy (data-parallel over B):
#  - numerator (gold path score): cheap gathers, computed on host in numpy.
#  - denominator logZ: forward algorithm as a *linear-domain* scaled scan on
#    device.  alpha_{l+1} = (alpha_l @ exp(selfT)) * exp(em_{l+1}), with a
#    per-partition max-rescale every R steps; the max is stored to a column
#    of an SBUF tile and logged/accumulated on host afterwards.
#  - exp(em) is precomputed on host; each core holds its whole [P, L*G*T]
#    emission slice in one SBUF tile, loaded via 4 parallel-queue DMAs.
#  - the 2048-step scan runs as a 254-iteration hardware loop (For_i) over
#    8-step blocks with peeled first/last blocks — keeps the program at
#    ~250 instructions so Tile scheduling + neuronxcc stay fast.
#  - each of the 8 cores handles B/8 = 256 sequences as [128 partitions, 2
#    groups]; host folds start_transitions into em[0], applies
#    end_transitions at the end, and sums.
#
# NOTE: build with Bacc (not bass.Bass) — its compile() runs
# generate_event_semaphores, which splits multi-wait sync_info to satisfy
# the TRN2 1-wait-per-instruction constraint; walrus rejects the module
# otherwise ("Too many sync wait commands").

L, B, T = 2048, 2048, 7
NCORES = 8
P = 128                       # partitions
G = (B // NCORES) // P        # 2 batch groups per core
R = 8                         # renormalize every R steps
NREN = L // R - 1             # 255 renorms (none after the last step)
GT = G * T                    # 14
GTT = G * T * T               # 98
TT = T * T                    # 49
NBLK = L // R                 # 256 blocks of 8 steps
BLKW = R * GT                 # 112 floats per block
EMW = L * GT                  # 28672 floats per partition

LAST_EXEC_NS = None


def _build_bass():
    import concourse.mybir as mybir
    from concourse.bacc import Bacc
    from concourse.bass import ts
    from concourse.tile import TileContext

    f32 = mybir.dt.float32
    Alu = mybir.AluOpType
    X = mybir.AxisListType.X

    nc = Bacc()
    em = nc.declare_dram_parameter("em", [P, EMW], f32, isOutput=False)
    e128 = nc.declare_dram_parameter("e128", [P, TT], f32, isOutput=False)
    alpha_out = nc.declare_dram_parameter("alpha_out", [P, GT], f32, isOutput=True)
    smax_out = nc.declare_dram_parameter("smax_out", [P, NREN], f32, isOutput=True)

    with TileContext(nc) as tc:
        with (
            tc.tile_pool(name="const", bufs=1) as cpool,
            tc.tile_pool(name="work", bufs=2) as wpool,
        ):
            E = cpool.tile([P, TT], f32, tag="E")
            nc.sync.dma_start(out=E[:, :], in_=e128[:, :])
            emall = cpool.tile([P, EMW], f32, tag="emall")
            cuts = [0, 3584, 10752, 17920, EMW]    # 4 parallel-queue loads
            for qi in range(4):
                nc.sync.dma_start(out=emall[:, cuts[qi]:cuts[qi + 1]],
                                  in_=em[:, cuts[qi]:cuts[qi + 1]])
            alpha = cpool.tile([P, GT], f32, tag="alpha")
            smx = cpool.tile([P, NREN], f32, tag="smx")
            rt = cpool.tile([P, 1], f32, tag="rt")
            e_b = E[:, :].rearrange("p (j i) -> p j i", j=T)

            def do_step(emslice, fold, adst):
                qf = wpool.tile([P, GTT], f32, tag="qf")
                q = wpool.tile([P, GT], f32, tag="q")
                for g in range(G):
                    a_b = alpha[:, g * T:(g + 1) * T].rearrange(
                        "p t -> p () t"
                    ).broadcast_to([P, T, T])
                    qf3 = qf[:, g * TT:(g + 1) * TT].rearrange(
                        "p (j i) -> p j i", j=T
                    )
                    if fold:
                        # fold in the pending 1/max rescale from the renorm
                        nc.vector.scalar_tensor_tensor(
                            out=qf3, in0=a_b, scalar=rt[:, 0:1], in1=e_b,
                            op0=Alu.mult, op1=Alu.mult,
                        )
                    else:
                        nc.vector.tensor_tensor(
                            out=qf3, in0=a_b, in1=e_b, op=Alu.mult
                        )
                nc.vector.tensor_reduce(
                    out=q[:, :],
                    in_=qf[:, :].rearrange("p (gj i) -> p gj i", i=T),
                    axis=X, op=Alu.add,
                )
                nc.vector.tensor_tensor(
                    out=adst, in0=q[:, :], in1=emslice, op=Alu.mult
                )

            def renorm(smx_col):
                nc.vector.reduce_max(out=smx_col, in_=alpha[:, :], axis=X)
                nc.vector.reciprocal(out=rt[:, :], in_=smx_col)

            # block 0 (peeled): init + steps 1..7 + renorm 0
            nc.vector.tensor_copy(out=alpha[:, :], in_=emall[:, 0:GT])
            for s in range(1, R):
                do_step(emall[:, s * GT:(s + 1) * GT], False, alpha[:, :])
            renorm(smx[:, 0:1])

            # blocks 1..254: hardware loop
            with tc.For_i(1, NBLK - 1) as it:
                base = emall[:, ts(it, BLKW)]
                do_step(base[:, 0:GT], True, alpha[:, :])
                for s in range(1, R):
                    do_step(base[:, s * GT:(s + 1) * GT], False, alpha[:, :])
                renorm(smx[:, ts(it, 1)])

            # block 255 (peeled): fold + steps, no trailing renorm
            base = emall[:, (NBLK - 1) * BLKW:NBLK * BLKW]
            do_step(base[:, 0:GT], True, alpha[:, :])
            for s in range(1, R):
                do_step(base[:, s * GT:(s + 1) * GT], False, alpha[:, :])

            nc.sync.dma_start(out=alpha_out[:, :], in_=alpha[:, :])
            nc.sync.dma_start(out=smax_out[:, :], in_=smx[:, :])
    nc.compile()
    return nc


def _device_logZ(emissions, start_transitions, end_transitions, self_transitions):
    """Returns logZ summed over all B, computed on 8 NeuronCores."""
    global LAST_EXEC_NS
    sys.path.insert(0, "/opt/trn_rl_repo")
    from concourse.bass_utils import run_bass_kernel_spmd

    t0 = time.time()
    # host relayout: em_dev[core, p, l*GT + g*T + t] = em[l, core*256+g*128+p, t]
    eml = np.asarray(emissions, dtype=np.float32).reshape(L, NCORES, G, P, T)
    em_dev = np.ascontiguousarray(eml.transpose(1, 3, 0, 2, 4))  # [core,P,L,G,T]
    em_dev[:, :, 0, 0, :] += np.asarray(start_transitions, np.float32)[None, None, :]
    np.exp(em_dev, out=em_dev)
    em_dev = em_dev.reshape(NCORES, P, EMW)
    E_np = np.exp(np.asarray(self_transitions, np.float64)).astype(np.float32).T
    E128 = np.broadcast_to(E_np.reshape(1, TT), (P, TT)).copy()
    t1 = time.time()

    nc = _build_bass()
    t2 = time.time()

    in_maps = [{"em": em_dev[i], "e128": E128} for i in range(NCORES)]
    # NTFF profiling under axon needs antenv.axon_hooks; only attempt a
    # traced run when it is importable (a failed traced run would force a
    # second full compile+execute).
    want_trace = False
    try:
        import antenv.axon_hooks  # noqa: F401
        want_trace = True
    except Exception:
        pass
    res = None
    if want_trace:
        try:
            res = run_bass_kernel_spmd(nc, in_maps, list(range(NCORES)),
                                       trace=True)
            LAST_EXEC_NS = res.exec_time_ns
        except Exception as e:
            print(f"[kernel] traced run failed ({e!r}); retrying without "
                  f"trace", file=sys.stderr)
            res = None
    if res is None:
        res = run_bass_kernel_spmd(nc, in_maps, list(range(NCORES)))
        LAST_EXEC_NS = None
    t3 = time.time()
    print(f"[kernel] relayout {t1-t0:.2f}s build {t2-t1:.2f}s run {t3-t2:.2f}s "
          f"exec_ns={LAST_EXEC_NS}", file=sys.stderr)

    logZ = 0.0
    eend = np.exp(np.asarray(end_transitions, np.float64))  # [t]
    for i in range(NCORES):
        af = res.results[i]["alpha_out"].astype(np.float64).reshape(P, G, T)
        sm = res.results[i]["smax_out"].astype(np.float64)  # [P, NREN]
        off = np.sum(np.log(sm), axis=1)                    # [P]
        logZ += np.sum(np.log(af @ eend) + off[:, None])
    return logZ


def _host_score(em, tags, qmask, mask_i, st, et, selfT, otherT):
    """Gold path score, summed over B (numpy, f32 gathers / f64 sums)."""
    contagion = qmask[1:] != qmask[:-1]
    em_tag = np.take_along_axis(em, tags[:, :, None], axis=2)[:, :, 0]   # [L,B] f32
    if contagion.any():
        trans_tag = np.where(contagion,
                             otherT[tags[:-1], tags[1:]],
                             selfT[tags[:-1], tags[1:]])
    else:
        trans_tag = selfT[tags[:-1], tags[1:]]
    score = np.sum(st[tags[0]], dtype=np.float64)
    score += np.sum(em_tag[0], dtype=np.float64)
    if np.all(mask_i[1:] != 0):
        score += np.sum(trans_tag, dtype=np.float64)
        score += np.sum(em_tag[1:], dtype=np.float64)
        score += np.sum(et[tags[-1]], dtype=np.float64)
    else:
        maskf = mask_i[1:].astype(np.float64)
        score += np.sum((trans_tag + em_tag[1:]) * maskf)
        seq_ends = mask_i.sum(axis=0) - 1
        score += np.sum(et[tags[seq_ends, np.arange(em.shape[1])]], dtype=np.float64)
    return float(score)


def _host_logZ_simple(em, st, et, selfT):
    """Vectorized scaled linear-domain scan (simple case: full mask, one
    speaker). f64, renorm every 16 steps."""
    Efwd = np.exp(np.asarray(selfT, np.float64))            # [i,j]
    e = np.exp(np.asarray(em, np.float64))                  # [L,B,T]
    alpha = np.exp(np.asarray(st, np.float64))[None, :] * e[0]
    off = np.zeros(alpha.shape[0])
    for l in range(1, em.shape[0]):
        alpha = (alpha @ Efwd) * e[l]
        if l % 16 == 0:
            m = alpha.max(axis=1)
            alpha /= m[:, None]
            off += np.log(m)
    fin = alpha * np.exp(np.asarray(et, np.float64))[None, :]
    return float(np.sum(np.log(fin.sum(axis=1)) + off))


def _host_logZ_general(em, qmask, mask_i, st, et, selfT, otherT):
    em = np.asarray(em, np.float64)
    contagion = qmask[1:] != qmask[:-1]
    any_cont = contagion.any(axis=1)
    all_mask = (mask_i != 0).all(axis=1)
    alpha = st[None, :] + em[0]
    for l in range(1, em.shape[0]):
        if any_cont[l - 1]:
            trans = np.where(contagion[l - 1][:, None, None], otherT[None], selfT[None])
            x = alpha[:, :, None] + trans
        else:
            x = alpha[:, :, None] + selfT[None]
        m = x.max(axis=1)
        new = np.log(np.exp(x - m[:, None, :]).sum(axis=1)) + m + em[l]
        if all_mask[l]:
            alpha = new
        else:
            alpha = np.where(mask_i[l][:, None] > 0, new, alpha)
    fin = alpha + et[None, :]
    mm = fin.max(axis=1)
    return float(np.sum(np.log(np.exp(fin - mm[:, None]).sum(axis=1)) + mm))


def kernel(emissions, tags, qmask, mask, start_transitions, end_transitions,
           self_transitions, other_transitions):
    emissions = np.asarray(emissions, dtype=np.float32)
    tags = np.asarray(tags)
    qmask = np.asarray(qmask)
    mask_i = np.asarray(mask)
    st = np.asarray(start_transitions, np.float64)
    et = np.asarray(end_transitions, np.float64)
    selfT = np.asarray(self_transitions, np.float64)
    otherT = np.asarray(other_transitions, np.float64)

    score = _host_score(emissions, tags, qmask, mask_i, st, et, selfT, otherT)

    simple = (not np.any(qmask[1:] != qmask[:-1])) and np.all(mask_i != 0)
    logZ = None
    if simple and emissions.shape == (L, B, T):
        try:
            logZ = _device_logZ(emissions, start_transitions, end_transitions,
                                self_transitions)
        except Exception as e:
            print(f"[kernel] device path failed ({e!r}); numpy fallback",
                  file=sys.stderr)
            logZ = None
    if logZ is None:
        if simple:
            logZ = _host_logZ_simple(emissions, st, et, selfT)
        else:
            logZ = _host_logZ_general(emissions, qmask, mask_i, st, et,
                                      selfT, otherT)

    return np.array(score - logZ, dtype=np.float32)


# revision 9
# speedup vs baseline: 15.5559x; 1.5815x over previous
import os
import sys
import time
import numpy as np

# nn_CRF loss on 8 NeuronCores: emissions [L,B,T], tags/qmask/mask [L,B],
# transitions T/TxT.  Returns scalar f32 sum_b (gold-path score - logZ).
#
# Strategy (data-parallel over B):
#  - numerator (gold path score): cheap gathers, computed on host in numpy.
#  - denominator logZ: forward algorithm as a *linear-domain* scaled scan on
#    device.  alpha_{l+1} = (alpha_l @ exp(selfT)) * exp(em_{l+1}), with a
#    per-partition max-rescale every R steps; the max is stored to a column
#    of an SBUF tile and logged/accumulated on host afterwards.
#  - exp(em) is precomputed on host; each core holds its whole [P, L*G*T]
#    emission slice in one SBUF tile, loaded via 4 parallel-queue DMAs.
#  - the 2048-step scan runs as a 254-iteration hardware loop (For_i) over
#    8-step blocks with peeled first/last blocks — keeps the program at
#    ~250 instructions so Tile scheduling + neuronxcc stay fast.
#  - each of the 8 cores handles B/8 = 256 sequences as [128 partitions, 2
#    groups]; host folds start_transitions into em[0], applies
#    end_transitions at the end, and sums.
#
# NOTE: build with Bacc (not bass.Bass) — its compile() runs
# generate_event_semaphores, which splits multi-wait sync_info to satisfy
# the TRN2 1-wait-per-instruction constraint; walrus rejects the module
# otherwise ("Too many sync wait commands").

L, B, T = 2048, 2048, 7
NCORES = 8
P = 128                       # partitions
G = (B // NCORES) // P        # 2 batch groups per core
R = 8                         # renormalize every R steps
NREN = L // R - 1             # 255 renorms (none after the last step)
GT = G * T                    # 14
GTT = G * T * T               # 98
TT = T * T                    # 49
NBLK = L // R                 # 256 blocks of 8 steps
BLKW = R * GT                 # 112 floats per block
EMW = L * GT                  # 28672 floats per partition

LAST_EXEC_NS = None


def _build_bass(use_bf16):
    import concourse.mybir as mybir
    from concourse.bacc import Bacc
    from concourse.bass import ts
    from concourse.tile import TileContext

    f32 = mybir.dt.float32
    emdt = mybir.dt.bfloat16 if use_bf16 else f32
    Alu = mybir.AluOpType
    X = mybir.AxisListType.X

    nc = Bacc()
    em = nc.declare_dram_parameter("em", [P, EMW], emdt, isOutput=False)
    e128 = nc.declare_dram_parameter("e128", [P, TT], f32, isOutput=False)
    alpha_out = nc.declare_dram_parameter("alpha_out", [P, GT], f32, isOutput=True)
    smax_out = nc.declare_dram_parameter("smax_out", [P, NREN], f32, isOutput=True)

    with TileContext(nc) as tc:
        with (
            tc.tile_pool(name="const", bufs=1) as cpool,
            tc.tile_pool(name="work", bufs=2) as wpool,
        ):
            E = cpool.tile([P, TT], f32, tag="E")
            nc.sync.dma_start(out=E[:, :], in_=e128[:, :])
            emall = cpool.tile([P, EMW], emdt, tag="emall")
            cuts = [0, 3584, 10752, 17920, EMW]    # 4 parallel-queue loads
            for qi in range(4):
                nc.sync.dma_start(out=emall[:, cuts[qi]:cuts[qi + 1]],
                                  in_=em[:, cuts[qi]:cuts[qi + 1]])
            alpha = cpool.tile([P, GT], f32, tag="alpha")
            smx = cpool.tile([P, NREN], f32, tag="smx")
            rt = cpool.tile([P, 1], f32, tag="rt")
            e_b = E[:, :].rearrange("p (j i) -> p j i", j=T)

            def do_step(emslice, fold, adst):
                qf = wpool.tile([P, GTT], f32, tag="qf")
                q = wpool.tile([P, GT], f32, tag="q")
                for g in range(G):
                    a_b = alpha[:, g * T:(g + 1) * T].rearrange(
                        "p t -> p () t"
                    ).broadcast_to([P, T, T])
                    qf3 = qf[:, g * TT:(g + 1) * TT].rearrange(
                        "p (j i) -> p j i", j=T
                    )
                    if fold:
                        # fold in the pending 1/max rescale from the renorm
                        nc.vector.scalar_tensor_tensor(
                            out=qf3, in0=a_b, scalar=rt[:, 0:1], in1=e_b,
                            op0=Alu.mult, op1=Alu.mult,
                        )
                    else:
                        nc.vector.tensor_tensor(
                            out=qf3, in0=a_b, in1=e_b, op=Alu.mult
                        )
                nc.vector.tensor_reduce(
                    out=q[:, :],
                    in_=qf[:, :].rearrange("p (gj i) -> p gj i", i=T),
                    axis=X, op=Alu.add,
                )
                nc.vector.tensor_tensor(
                    out=adst, in0=q[:, :], in1=emslice, op=Alu.mult
                )

            def renorm(smx_col):
                nc.vector.reduce_max(out=smx_col, in_=alpha[:, :], axis=X)
                nc.vector.reciprocal(out=rt[:, :], in_=smx_col)

            # block 0 (peeled): init + steps 1..7 + renorm 0
            nc.vector.tensor_copy(out=alpha[:, :], in_=emall[:, 0:GT])
            for s in range(1, R):
                do_step(emall[:, s * GT:(s + 1) * GT], False, alpha[:, :])
            renorm(smx[:, 0:1])

            # blocks 1..254: hardware loop
            with tc.For_i(1, NBLK - 1) as it:
                base = emall[:, ts(it, BLKW)]
                do_step(base[:, 0:GT], True, alpha[:, :])
                for s in range(1, R):
                    do_step(base[:, s * GT:(s + 1) * GT], False, alpha[:, :])
                renorm(smx[:, ts(it, 1)])

            # block 255 (peeled): fold + steps, no trailing renorm
            base = emall[:, (NBLK - 1) * BLKW:NBLK * BLKW]
            do_step(base[:, 0:GT], True, alpha[:, :])
            for s in range(1, R):
                do_step(base[:, s * GT:(s + 1) * GT], False, alpha[:, :])

            nc.sync.dma_start(out=alpha_out[:, :], in_=alpha[:, :])
            nc.sync.dma_start(out=smax_out[:, :], in_=smx[:, :])
    nc.compile()
    return nc


def _device_logZ(emissions, start_transitions, end_transitions, self_transitions):
    """Returns logZ summed over all B, computed on 8 NeuronCores."""
    global LAST_EXEC_NS
    sys.path.insert(0, "/opt/trn_rl_repo")
    from concourse.bass_utils import run_bass_kernel_spmd

    use_bf16 = False
    try:
        import ml_dtypes
        use_bf16 = True
    except Exception:
        pass

    t0 = time.time()
    # host relayout: em_dev[core, p, l*GT + g*T + t] = em[l, core*256+g*128+p, t]
    eml = np.asarray(emissions, dtype=np.float32).reshape(L, NCORES, G, P, T)
    em_dev = np.ascontiguousarray(eml.transpose(1, 3, 0, 2, 4))  # [core,P,L,G,T]
    em_dev[:, :, 0, 0, :] += np.asarray(start_transitions, np.float32)[None, None, :]
    np.exp(em_dev, out=em_dev)
    em_dev = em_dev.reshape(NCORES, P, EMW)
    if use_bf16:
        em_dev = em_dev.astype(ml_dtypes.bfloat16)
    E_np = np.exp(np.asarray(self_transitions, np.float64)).astype(np.float32).T
    E128 = np.broadcast_to(E_np.reshape(1, TT), (P, TT)).copy()
    t1 = time.time()

    nc = _build_bass(use_bf16)
    t2 = time.time()

    in_maps = [{"em": em_dev[i], "e128": E128} for i in range(NCORES)]
    # NTFF profiling under axon needs antenv.axon_hooks; only attempt a
    # traced run when it is importable (a failed traced run would force a
    # second full compile+execute).
    want_trace = False
    try:
        import antenv.axon_hooks  # noqa: F401
        want_trace = True
    except Exception:
        pass
    res = None
    if want_trace:
        try:
            res = run_bass_kernel_spmd(nc, in_maps, list(range(NCORES)),
                                       trace=True)
            LAST_EXEC_NS = res.exec_time_ns
        except Exception as e:
            print(f"[kernel] traced run failed ({e!r}); retrying without "
                  f"trace", file=sys.stderr)
            res = None
    if res is None:
        res = run_bass_kernel_spmd(nc, in_maps, list(range(NCORES)))
        LAST_EXEC_NS = None
    t3 = time.time()
    print(f"[kernel] relayout {t1-t0:.2f}s build {t2-t1:.2f}s run {t3-t2:.2f}s "
          f"exec_ns={LAST_EXEC_NS}", file=sys.stderr)

    logZ = 0.0
    eend = np.exp(np.asarray(end_transitions, np.float64))  # [t]
    for i in range(NCORES):
        af = res.results[i]["alpha_out"].astype(np.float64).reshape(P, G, T)
        sm = res.results[i]["smax_out"].astype(np.float64)  # [P, NREN]
        off = np.sum(np.log(sm), axis=1)                    # [P]
        logZ += np.sum(np.log(af @ eend) + off[:, None])
    return logZ


def _host_score(em, tags, qmask, mask_i, st, et, selfT, otherT):
    """Gold path score, summed over B (numpy, f32 gathers / f64 sums)."""
    contagion = qmask[1:] != qmask[:-1]
    em_tag = np.take_along_axis(em, tags[:, :, None], axis=2)[:, :, 0]   # [L,B] f32
    if contagion.any():
        trans_tag = np.where(contagion,
                             otherT[tags[:-1], tags[1:]],
                             selfT[tags[:-1], tags[1:]])
    else:
        trans_tag = selfT[tags[:-1], tags[1:]]
    score = np.sum(st[tags[0]], dtype=np.float64)
    score += np.sum(em_tag[0], dtype=np.float64)
    if np.all(mask_i[1:] != 0):
        score += np.sum(trans_tag, dtype=np.float64)
        score += np.sum(em_tag[1:], dtype=np.float64)
        score += np.sum(et[tags[-1]], dtype=np.float64)
    else:
        maskf = mask_i[1:].astype(np.float64)
        score += np.sum((trans_tag + em_tag[1:]) * maskf)
        seq_ends = mask_i.sum(axis=0) - 1
        score += np.sum(et[tags[seq_ends, np.arange(em.shape[1])]], dtype=np.float64)
    return float(score)


def _host_logZ_simple(em, st, et, selfT):
    """Vectorized scaled linear-domain scan (simple case: full mask, one
    speaker). f64, renorm every 16 steps."""
    Efwd = np.exp(np.asarray(selfT, np.float64))            # [i,j]
    e = np.exp(np.asarray(em, np.float64))                  # [L,B,T]
    alpha = np.exp(np.asarray(st, np.float64))[None, :] * e[0]
    off = np.zeros(alpha.shape[0])
    for l in range(1, em.shape[0]):
        alpha = (alpha @ Efwd) * e[l]
        if l % 16 == 0:
            m = alpha.max(axis=1)
            alpha /= m[:, None]
            off += np.log(m)
    fin = alpha * np.exp(np.asarray(et, np.float64))[None, :]
    return float(np.sum(np.log(fin.sum(axis=1)) + off))


def _host_logZ_general(em, qmask, mask_i, st, et, selfT, otherT):
    em = np.asarray(em, np.float64)
    contagion = qmask[1:] != qmask[:-1]
    any_cont = contagion.any(axis=1)
    all_mask = (mask_i != 0).all(axis=1)
    alpha = st[None, :] + em[0]
    for l in range(1, em.shape[0]):
        if any_cont[l - 1]:
            trans = np.where(contagion[l - 1][:, None, None], otherT[None], selfT[None])
            x = alpha[:, :, None] + trans
        else:
            x = alpha[:, :, None] + selfT[None]
        m = x.max(axis=1)
        new = np.log(np.exp(x - m[:, None, :]).sum(axis=1)) + m + em[l]
        if all_mask[l]:
            alpha = new
        else:
            alpha = np.where(mask_i[l][:, None] > 0, new, alpha)
    fin = alpha + et[None, :]
    mm = fin.max(axis=1)
    return float(np.sum(np.log(np.exp(fin - mm[:, None]).sum(axis=1)) + mm))


def kernel(emissions, tags, qmask, mask, start_transitions, end_transitions,
           self_transitions, other_transitions):
    emissions = np.asarray(emissions, dtype=np.float32)
    tags = np.asarray(tags)
    qmask = np.asarray(qmask)
    mask_i = np.asarray(mask)
    st = np.asarray(start_transitions, np.float64)
    et = np.asarray(end_transitions, np.float64)
    selfT = np.asarray(self_transitions, np.float64)
    otherT = np.asarray(other_transitions, np.float64)

    score = _host_score(emissions, tags, qmask, mask_i, st, et, selfT, otherT)

    simple = (not np.any(qmask[1:] != qmask[:-1])) and np.all(mask_i != 0)
    logZ = None
    if simple and emissions.shape == (L, B, T):
        try:
            logZ = _device_logZ(emissions, start_transitions, end_transitions,
                                self_transitions)
        except Exception as e:
            print(f"[kernel] device path failed ({e!r}); numpy fallback",
                  file=sys.stderr)
            logZ = None
    if logZ is None:
        if simple:
            logZ = _host_logZ_simple(emissions, st, et, selfT)
        else:
            logZ = _host_logZ_general(emissions, qmask, mask_i, st, et,
                                      selfT, otherT)

    return np.array(score - logZ, dtype=np.float32)
